# revision 1
# baseline (speedup 1.0000x reference)
"""Trainium2 Bass kernel for nn_AAttn (area attention block), SPMD over 8 cores.

Reference computation (eval-mode):
  qkv = BN(conv1x1(x, w_qkv))                       # [B,3C,H,W]
  per (batch, area) of B*AREA=8: per head (8, hd=32):
     S = q^T k / sqrt(hd); A = softmax(S, -1); o = v @ A^T
  pe  = BN(dwconv7(v2))
  out = BN(conv1x1(o + pe, w_proj))

Sharding: core i <-> (b, a) = (i//4, i%4) — one area per core (data parallel,
no collectives). Each core gets x rows [16a-3, 16a+19) zero-padded (halo for
the 7x7 depthwise conv), computes its 16 output rows.

Host folds all BN affines into conv weights/biases, permutes channels into
head-packed order (c = h*32 + d, which is exactly the reference's o/v2
channel order), folds 1/sqrt(hd) into Wq and bias_pe into b_proj.
"""

import os
import sys
import numpy as np

sys.path.insert(0, "/opt/trn_rl_repo")

import ml_dtypes  # noqa: E402

import concourse.bass as bass  # noqa: E402
from concourse import bacc, mybir  # noqa: E402
from concourse.tile import TileContext  # noqa: E402
from concourse.bass_utils import run_bass_kernel_spmd  # noqa: E402
from concourse.masks import make_identity  # noqa: E402

F32 = mybir.dt.float32
F32R = mybir.dt.float32r
BF16 = mybir.dt.bfloat16

EPS = 1e-5
HEADS = 8
AREA = 4
C = 256
HD = 32          # head dim
B = 2
H = W = 64
ROWS = 16        # output rows per core
HALO = 3
HR = ROWS + 2 * HALO       # 22 halo rows
HW = W + 2 * HALO          # 70 halo cols
NSP = HR * HW              # 1540 halo spatial
NCEN = ROWS * W            # 1024 central spatial
G = 2                      # head groups of 4 (128 channels each)

ALL_TAPS = [(dy, dx) for dy in range(-3, 4) for dx in range(-3, 4)]
# DVE-eligible taps need even (bf16 4-byte aligned) flat offsets:
# offset = (3+dy)*70 + (3+dx); 70 even => parity = (3+dx) parity => dx odd.
N_DVE_TAPS = int(os.environ.get("AATTN_DVE_TAPS", "20"))
N_GP_TAPS = int(os.environ.get("AATTN_GP_TAPS", "0"))
_dve_cand = [t for t in ALL_TAPS if (3 + t[1]) % 2 == 0]
TAPS_DVE = _dve_cand[:N_DVE_TAPS]
_rest = [t for t in ALL_TAPS if t not in TAPS_DVE]
TAPS_GP = _rest[:N_GP_TAPS]
TAPS_PE = [t for t in _rest if t not in TAPS_GP]


def build_nc():
    nc = bacc.Bacc("TRN2", target_bir_lowering=False, debug=False, num_devices=8)

    d_x = nc.declare_dram_parameter("x_local", [2, 128, NSP], BF16, isOutput=False)
    d_mask = nc.declare_dram_parameter("mask", [1, NSP], F32, isOutput=False)
    d_wq = nc.declare_dram_parameter("wqT", [2, 128, C], BF16, isOutput=False)
    d_wk = nc.declare_dram_parameter("wkT", [2, 128, C], BF16, isOutput=False)
    d_wv = nc.declare_dram_parameter("wvT", [2, 128, C], BF16, isOutput=False)
    d_wp = nc.declare_dram_parameter("wpT", [2, 128, C], BF16, isOutput=False)
    d_bias = nc.declare_dram_parameter("biases", [2, 128, 4], F32, isOutput=False)
    n_pe = max(len(TAPS_PE), 1)
    d_diag = nc.declare_dram_parameter("diag_pe", [n_pe, 2, 128, 128], BF16,
                                       isOutput=False)
    n_dve = max(len(TAPS_DVE), 1)
    d_wdve = nc.declare_dram_parameter("w_dve", [2, 128, n_dve], F32, isOutput=False)
    n_gp = max(len(TAPS_GP), 1)
    d_wgp = nc.declare_dram_parameter("w_gp", [2, 128, n_gp], F32, isOutput=False)
    d_bvb = nc.declare_dram_parameter("bvb", [2, 1, 128], F32, isOutput=False)
    d_out = nc.declare_dram_parameter("out", [2, 128, NCEN], F32, isOutput=True)

    with TileContext(nc) as tc:
        _build_body(nc, tc, d_x, d_mask, d_wq, d_wk, d_wv, d_wp, d_bias,
                    d_diag, d_wdve, d_wgp, d_bvb, d_out)

    nc.compile()
    return nc


def _build_body(nc, tc, d_x, d_mask, d_wq, d_wk, d_wv, d_wp, d_bias,
                d_diag, d_wdve, d_wgp, d_bvb, d_out):
    from contextlib import ExitStack

    ctx = ExitStack()
    with ctx:
        persist = ctx.enter_context(tc.tile_pool(name="persist", bufs=1))

        # ---- static SBUF tensors ----
        x_sb = persist.tile([128, 2, NSP], BF16, tag="x")
        mask_sb = persist.tile([128, NSP], F32, tag="mask")
        wq_sb = persist.tile([128, 2, C], BF16, tag="wq")
        wk_sb = persist.tile([128, 2, C], BF16, tag="wk")
        wv_sb = persist.tile([128, 2, C], BF16, tag="wv")
        wp_sb = persist.tile([128, 2, C], BF16, tag="wp")
        b_sb = persist.tile([128, 2, 4], F32, tag="bias")
        diag_sb = None
        if TAPS_PE:
            diag_sb = persist.tile([128, len(TAPS_PE), 2, 128], BF16,
                                   tag="diag", name="diag_sb")
        wdve_sb = persist.tile([128, 2, max(len(TAPS_DVE), 1)], F32, tag="wdve")
        wgp_sb = persist.tile([128, 2, max(len(TAPS_GP), 1)], F32, tag="wgp")
        accgp_sb = persist.tile([128, 2, NCEN], BF16, tag="accgp")
        ones_sb = persist.tile([128, 32], BF16, tag="ones")

        q_sb = persist.tile([128, 2, NCEN], BF16, tag="q")
        k_sb = persist.tile([128, 2, NCEN], BF16, tag="k")
        v_sb = persist.tile([128, 2, NSP], F32, tag="v")
        xcen_sb = persist.tile([128, 2, NCEN], BF16, tag="xcen")
        bvb_sb = persist.tile([128, 2, 128], F32, tag="bvb")
        vbf_sb = persist.tile([128, 2, NSP], BF16, tag="vbf")
        vT_sb = persist.tile([128, 2, 8, 128], BF16, tag="vT")
        accd_sb = persist.tile([128, 2, NCEN], BF16, tag="accd")
        pin_sb = persist.tile([128, 2, NCEN], BF16, tag="pin")
        pesb_sb = persist.tile([128, 2, 2, 512], F32, tag="pesb")
        out_sb = persist.tile([128, 2, NCEN], F32, tag="outsb")

        # ---- input DMAs ----
        for t in range(2):
            nc.sync.dma_start(out=x_sb[:, t, :], in_=d_x[t])
        nc.sync.dma_start(out=mask_sb[:], in_=d_mask[:].partition_broadcast(128))
        for dst, src in ((wq_sb, d_wq), (wk_sb, d_wk), (wv_sb, d_wv), (wp_sb, d_wp)):
            for t in range(2):
                nc.sync.dma_start(out=dst[:, t, :], in_=src[t])
        for t in range(2):
            nc.sync.dma_start(out=b_sb[:, t, :], in_=d_bias[t])
        if TAPS_PE:
            for t in range(2):
                nc.gpsimd.dma_start(
                    out=diag_sb[:, :, t, :],
                    in_=d_diag.rearrange("j t p c -> p j t c")[:, :, t, :])
        for t in range(2):
            nc.gpsimd.dma_start(out=wdve_sb[:, t, :], in_=d_wdve[t])
            nc.gpsimd.dma_start(out=wgp_sb[:, t, :], in_=d_wgp[t])

        nc.vector.memset(ones_sb[:], 1.0)
        for t in range(2):
            nc.sync.dma_start(
                out=bvb_sb[:, t, :],
                in_=d_bvb[t].partition_broadcast(128))
        for t in range(2):
            x3t = x_sb[:, t, :].rearrange("p (r c) -> p r c", c=HW)
            nc.vector.tensor_copy(
                xcen_sb[:, t, :],
                x3t[:, HALO: HALO + ROWS, HALO: HALO + W])

        def x3(t):
            return x_sb[:, t, :].rearrange("p (r c) -> p r c", c=HW)

        def cen(ap3, q8):  # 512-col central chunk (8 rows) of a [p, 22, 70] view
            return ap3[:, HALO + 8 * q8: HALO + 8 * (q8 + 1), HALO: HALO + W]

        # =========== phase 1: qkv ===========
        with tc.tile_pool(name="ps_qk", bufs=2, space="PSUM") as ps_qk, \
             tc.tile_pool(name="ps_v", bufs=1, space="PSUM") as ps_v:

            # q, k: central spatial only.  bias index 0 = q, 1 = k, 2 = v, 3 = proj
            for g in range(G):
                for (w_t, o_t, bidx) in ((wq_sb, q_sb, 0), (wk_sb, k_sb, 1)):
                    psum = ps_qk.tile([128, NCEN], F32, tag="qk")
                    for ch in range(2):
                        for kt in range(2):
                            nc.tensor.matmul(
                                psum[:, 512 * ch: 512 * (ch + 1)],
                                lhsT=(w_t[:, kt, 128 * g: 128 * (g + 1)]),
                                rhs=(cen(x3(kt), ch)),
                                start=(kt == 0), stop=(kt == 1))
                    nc.vector.tensor_scalar_add(
                        o_t[:, g, :], psum[:], b_sb[:, g, bidx: bidx + 1])

            # v: full halo spatial, then mask * (psum + bias)
            for g in range(G):
                psum = ps_v.tile([128, NSP], F32, tag="v")
                for (c0, cn) in ((0, 512), (512, 512), (1024, 512), (1536, 4)):
                    for kt in range(2):
                        nc.tensor.matmul(
                            psum[:, c0: c0 + cn],
                            lhsT=(wv_sb[:, kt, 128 * g: 128 * (g + 1)]),
                            rhs=(x_sb[:, kt, c0: c0 + cn]),
                            start=(kt == 0), stop=(kt == 1))
                nc.vector.scalar_tensor_tensor(
                    out=v_sb[:, g, :], in0=psum[:], scalar=b_sb[:, g, 2: 3],
                    in1=mask_sb[:],
                    op0=mybir.AluOpType.add, op1=mybir.AluOpType.mult)
                nc.vector.tensor_copy(vbf_sb[:, g, :], v_sb[:, g, :])

        def v3(t):
            return v_sb[:, t, :].rearrange("p (r c) -> p r c", c=HW)

        def vbf3(t):
            return vbf_sb[:, t, :].rearrange("p (r c) -> p r c", c=HW)

        # =========== phase 1b: v^T ===========
        with tc.tile_pool(name="ps_t", bufs=2, space="PSUM") as ps_t:
            # v^T directly: vT[m, d4] = sum_c x[c, m] WvT[c, d4] + bv
            for g in range(G):
                for mt in range(8):
                    pst = ps_t.tile([128, 128], F32, tag="tp")
                    for kt in range(2):
                        nc.tensor.matmul(
                            pst[:],
                            lhsT=xcen_sb[:, kt, 128 * mt: 128 * (mt + 1)],
                            rhs=wv_sb[:, kt, 128 * g: 128 * (g + 1)],
                            start=(kt == 0), stop=(kt == 1))
                    nc.vector.scalar_tensor_tensor(
                        out=vT_sb[:, g, mt, :], in0=pst[:], scalar=1.0,
                        in1=bvb_sb[:, g, :],
                        op0=mybir.AluOpType.bypass, op1=mybir.AluOpType.add)

        # =========== phase 2: attention + dwconv ===========
        with tc.tile_pool(name="ps_s", bufs=1, space="PSUM") as ps_s, \
             tc.tile_pool(name="ps_o", bufs=1, space="PSUM") as ps_o, \
             tc.tile_pool(name="ps_d", bufs=1, space="PSUM") as ps_d, \
             tc.tile_pool(name="ps_pe", bufs=1, space="PSUM") as ps_pe, \
             tc.tile_pool(name="ps_p", bufs=1, space="PSUM") as ps_p, \
             tc.tile_pool(name="at", bufs=3) as at_pool, \
             tc.tile_pool(name="small", bufs=4) as small:

            on_sb = small.tile([128, 2, NCEN], F32, tag="on", name="on_sb")

            # dwconv DVE taps (independent; scheduler overlaps with attention)
            for t in range(G):
                nc.vector.memset(accd_sb[:, t, :], 0.0)
            for t in range(G):
                for j, (dy, dx) in enumerate(TAPS_DVE):
                    win = vbf3(t)[:, HALO + dy: HALO + dy + ROWS,
                                  HALO + dx: HALO + dx + W]
                    nc.vector.scalar_tensor_tensor(
                        out=accd_sb[:, t, :], in0=win,
                        scalar=wdve_sb[:, t, j: j + 1],
                        in1=accd_sb[:, t, :],
                        op0=mybir.AluOpType.mult, op1=mybir.AluOpType.add)

            # dwconv GPSIMD taps (gpsimd is otherwise idle); Pool has no
            # scalar_tensor_tensor codegen, so use mul + add pairs.
            if TAPS_GP:
                tmp_gp = small.tile([128, 2, NCEN], BF16, tag="tmpgp",
                                    name="tmp_gp", bufs=1)
                for t in range(G):
                    nc.gpsimd.memset(accgp_sb[:, t, :], 0.0)
                for t in range(G):
                    for j, (dy, dx) in enumerate(TAPS_GP):
                        win = vbf3(t)[:, HALO + dy: HALO + dy + ROWS,
                                      HALO + dx: HALO + dx + W]
                        nc.gpsimd.tensor_scalar(
                            out=tmp_gp[:, t, :], in0=win,
                            scalar1=wgp_sb[:, t, j: j + 1], scalar2=None,
                            op0=mybir.AluOpType.mult)
                        nc.gpsimd.tensor_add(
                            accgp_sb[:, t, :], accgp_sb[:, t, :],
                            tmp_gp[:, t, :])

            n_pe_taps = len(TAPS_PE)

            def s_mm(s_ps, g, ch, mt, i):
                nc.tensor.matmul(
                    s_ps[:, i, :],
                    lhsT=k_sb[32 * i: 32 * (i + 1), g,
                              128 * mt: 128 * (mt + 1)],
                    rhs=q_sb[32 * i: 32 * (i + 1), g,
                             512 * ch: 512 * (ch + 1)],
                    start=True, stop=True, skip_group_check=True,
                    tile_position=(32 * i, 0))

            def attn_group(g, ch):
                o_ps = ps_o.tile([128, 512], F32, tag="o", name="o_ps")
                d_ps = ps_d.tile([128, 512], F32, tag="d", name="d_ps")
                # software pipeline over mt: emit S(t+1) before O/D(t)
                # so the PE prefers the exp-critical S matmuls.
                ats = {}
                for mt in range(8):
                    s_ps = ps_s.tile([128, 4, 512], F32, tag="s", name="s_ps")
                    for i in range(4):
                        s_mm(s_ps, g, ch, mt, i)
                    at = at_pool.tile([128, 4, 512], BF16, tag="at", name="at")
                    nc.scalar.activation(at[:], s_ps[:],
                                         mybir.ActivationFunctionType.Exp)
                    ats[mt] = at
                for mt in range(8):
                    at = ats[mt]
                    for i in range(4):
                        nc.tensor.matmul(
                            o_ps[32 * i: 32 * (i + 1), :],
                            lhsT=vT_sb[:, g, mt, 32 * i: 32 * (i + 1)],
                            rhs=at[:, i, :],
                            start=(mt == 0), stop=(mt == 7),
                            skip_group_check=True,
                            tile_position=(0, 32 * i))
                        nc.tensor.matmul(
                            d_ps[32 * i: 32 * (i + 1), :],
                            lhsT=ones_sb[:, 0:32],
                            rhs=at[:, i, :],
                            start=(mt == 0), stop=(mt == 7),
                            skip_group_check=True,
                            tile_position=(0, 32 * i))

                # normalize O by 1/denominator -> on_sb (sbuf)
                r_sb = small.tile([128, 512], F32, tag="r", name="r_sb")
                nc.vector.reciprocal_approx_fast(out=r_sb[:], in_=d_ps[:])
                nc.vector.scalar_tensor_tensor(
                    out=on_sb[:, g, 512 * ch: 512 * (ch + 1)],
                    in0=o_ps[:], scalar=1.0, in1=r_sb[:],
                    op0=mybir.AluOpType.bypass, op1=mybir.AluOpType.mult)

            def dw_chain(t, ch):
                # dwconv PE tap chain; drains to SBUF WITHOUT needing o_n so
                # the psum bank frees early and the combine can happen later
                pe_d = pesb_sb[:, t, ch, :]
                accd3 = accd_sb[:, t, 512 * ch: 512 * (ch + 1)]
                if n_pe_taps:
                    pe_ps = ps_pe.tile([128, 512], F32, tag="pe", name="pe_ps")
                    for j, (dy, dx) in enumerate(TAPS_PE):
                        win = vbf3(t)[:, HALO + dy + 8 * ch:
                                      HALO + dy + 8 * (ch + 1),
                                      HALO + dx: HALO + dx + W]
                        nc.tensor.matmul(
                            pe_ps[:], lhsT=diag_sb[:, j, t, :], rhs=win,
                            start=(j == 0), stop=(j == n_pe_taps - 1))
                    if TAPS_DVE:
                        nc.vector.scalar_tensor_tensor(
                            out=pe_d, in0=pe_ps[:], scalar=1.0, in1=accd3,
                            op0=mybir.AluOpType.bypass,
                            op1=mybir.AluOpType.add)
                    else:
                        nc.vector.tensor_copy(pe_d, pe_ps[:])
                else:
                    nc.vector.tensor_copy(pe_d, accd3)

            def pin_combine(t, ch):
                nc.vector.tensor_add(
                    pin_sb[:, t, 512 * ch: 512 * (ch + 1)],
                    on_sb[:, t, 512 * ch: 512 * (ch + 1)],
                    pesb_sb[:, t, ch, :])

            def proj_chunk(ch):
                for o in range(G):
                    psum = ps_p.tile([128, 512], F32, tag="p", name="p_ps")
                    for t in range(2):
                        nc.tensor.matmul(
                            psum[:],
                            lhsT=wp_sb[:, t, 128 * o: 128 * (o + 1)],
                            rhs=pin_sb[:, t, 512 * ch: 512 * (ch + 1)],
                            start=(t == 0), stop=(t == 1))
                    nc.vector.tensor_scalar_add(
                        out_sb[:, o, 512 * ch: 512 * (ch + 1)], psum[:],
                        b_sb[:, o, 3: 4])
                    nc.sync.dma_start(
                        out=d_out[o, :, 512 * ch: 512 * (ch + 1)],
                        in_=out_sb[:, o, 512 * ch: 512 * (ch + 1)])

            attn_group(0, 0)
            attn_group(0, 1)
            attn_group(1, 0)
            attn_group(1, 1)
            dw_chain(0, 0)
            dw_chain(1, 0)
            dw_chain(0, 1)
            dw_chain(1, 1)
            pin_combine(0, 0)
            pin_combine(1, 0)
            proj_chunk(0)
            pin_combine(0, 1)
            pin_combine(1, 1)
            proj_chunk(1)


# ---------------------------------------------------------------------------
# host side
# ---------------------------------------------------------------------------

_NC_CACHE = {}


def _get_nc():
    if "nc" not in _NC_CACHE:
        _NC_CACHE["nc"] = build_nc()
    return _NC_CACHE["nc"]


def _prep_shared(w_qkv, g_qkv, b_qkv, m_qkv, var_qkv,
                 w_pe, g_pe, b_pe, m_pe, var_pe,
                 w_proj, g_proj, b_proj, m_proj, var_proj):
    f32 = np.float32
    s_qkv = (g_qkv / np.sqrt(var_qkv + EPS)).astype(f32)
    Wall = (w_qkv * s_qkv[:, None]).astype(f32)
    ball = (b_qkv - m_qkv * s_qkv).astype(f32)

    hs = np.arange(HEADS)
    perm_q = (hs[:, None] * 96 + np.arange(HD)[None, :]).reshape(-1)
    perm_k = perm_q + HD
    perm_v = perm_q + 2 * HD

    sc = f32(1.0 / np.sqrt(HD))
    Wq = Wall[perm_q] * sc
    bq = ball[perm_q] * sc
    Wk = Wall[perm_k]
    bk = ball[perm_k]
    Wv = Wall[perm_v]
    bv = ball[perm_v]

    s_pe = (g_pe / np.sqrt(var_pe + EPS)).astype(f32)
    wpe = (w_pe[:, 0] * s_pe[:, None, None]).astype(f32)      # [256,7,7]
    bpe = (b_pe - m_pe * s_pe).astype(f32)

    s_p = (g_proj / np.sqrt(var_proj + EPS)).astype(f32)
    Wp = (w_proj * s_p[:, None]).astype(f32)                  # [256(o),256(c=h*32+d)]
    bp = (b_proj - m_proj * s_p + Wp @ bpe).astype(f32)

    def kt(wT):  # [256(c_in),256(c_out)] -> [2,128,256]
        return np.ascontiguousarray(wT.reshape(2, 128, wT.shape[1]))

    bf = ml_dtypes.bfloat16
    d = {
        "wqT": kt(Wq.T).astype(bf),
        "wkT": kt(Wk.T).astype(bf),
        "wvT": kt(Wv.T).astype(bf),
        "wpT": kt(Wp.T).astype(bf),
    }
    biases = np.zeros((2, 128, 4), f32)
    for t in range(2):
        biases[t, :, 0] = bq[128 * t: 128 * (t + 1)]
        biases[t, :, 1] = bk[128 * t: 128 * (t + 1)]
        biases[t, :, 2] = bv[128 * t: 128 * (t + 1)]
        biases[t, :, 3] = bp[128 * t: 128 * (t + 1)]
    d["biases"] = biases

    wpe_flat = wpe.reshape(256, 49)
    n_pe = max(len(TAPS_PE), 1)
    diag = np.zeros((n_pe, 2, 128, 128), np.float32)
    for j, (dy, dx) in enumerate(TAPS_PE):
        tap = (dy + 3) * 7 + (dx + 3)
        for t in range(2):
            np.fill_diagonal(diag[j, t], wpe_flat[128 * t: 128 * (t + 1), tap])
    d["diag_pe"] = diag.astype(ml_dtypes.bfloat16)

    n_dve = max(len(TAPS_DVE), 1)
    wdve = np.zeros((2, 128, n_dve), f32)
    for j, (dy, dx) in enumerate(TAPS_DVE):
        tap = (dy + 3) * 7 + (dx + 3)
        for t in range(2):
            wdve[t, :, j] = wpe_flat[128 * t: 128 * (t + 1), tap]
    d["w_dve"] = wdve
    n_gp = max(len(TAPS_GP), 1)
    wgp = np.zeros((2, 128, n_gp), f32)
    for j, (dy, dx) in enumerate(TAPS_GP):
        tap = (dy + 3) * 7 + (dx + 3)
        for t in range(2):
            wgp[t, :, j] = wpe_flat[128 * t: 128 * (t + 1), tap]
    d["w_gp"] = wgp
    d["bvb"] = np.ascontiguousarray(bv.reshape(2, 1, 128)).astype(f32)
    return d


def _prep_core(x, core):
    b, a = divmod(core, 4)
    f32 = np.float32
    xl = np.zeros((C, HR, HW), f32)
    r0 = 16 * a - HALO
    r1 = 16 * a + ROWS + HALO
    s0, s1 = max(r0, 0), min(r1, H)
    xl[:, s0 - r0: s1 - r0, HALO: HALO + W] = x[b, :, s0:s1, :]

    mask = np.zeros((HR, HW), f32)
    mask[s0 - r0: s1 - r0, HALO: HALO + W] = 1.0
    return (np.ascontiguousarray(xl.reshape(2, 128, NSP)).astype(ml_dtypes.bfloat16),
            np.ascontiguousarray(mask.reshape(1, NSP)))


def kernel(**inputs):
    x = np.asarray(inputs["x"], np.float32)
    shared = _prep_shared(
        *[np.asarray(inputs[k], np.float32) for k in (
            "w_qkv", "g_qkv", "b_qkv", "m_qkv", "var_qkv",
            "w_pe", "g_pe", "b_pe", "m_pe", "var_pe",
            "w_proj", "g_proj", "b_proj", "m_proj", "var_proj")])

    in_maps = []
    for core in range(8):
        xl, mask = _prep_core(x, core)
        m = dict(shared)
        m["x_local"] = xl
        m["mask"] = mask
        in_maps.append(m)

    nc = _get_nc()
    outs = None
    last_exc = None
    for _attempt in range(3):
        try:
            res = run_bass_kernel_spmd(nc, in_maps, core_ids=list(range(8)))
            outs = res.results
            break
        except Exception as e:  # intermittent device errors: retry
            last_exc = e
            import time
            time.sleep(3)
    if outs is None:
        raise last_exc

    y = np.zeros((B, C, H, W), np.float32)
    for core in range(8):
        b, a = divmod(core, 4)
        o = np.asarray(outs[core]["out"], np.float32).reshape(C, ROWS, W)
        y[b, :, 16 * a: 16 * a + ROWS, :] = o
    return y


if __name__ == "__main__":
    rng = np.random.default_rng(0)
    ins = {
        "x": rng.standard_normal((2, C, H, W)).astype(np.float32),
        "w_qkv": (rng.standard_normal((768, 256)) * 0.05).astype(np.float32),
        "g_qkv": rng.uniform(size=768).astype(np.float32),
        "b_qkv": (rng.standard_normal(768) * 0.05).astype(np.float32),
        "m_qkv": (rng.standard_normal(768) * 0.05).astype(np.float32),
        "var_qkv": rng.uniform(size=768).astype(np.float32),
        "w_pe": (rng.standard_normal((256, 1, 7, 7)) * 0.05).astype(np.float32),
        "g_pe": rng.uniform(size=256).astype(np.float32),
        "b_pe": (rng.standard_normal(256) * 0.05).astype(np.float32),
        "m_pe": (rng.standard_normal(256) * 0.05).astype(np.float32),
        "var_pe": rng.uniform(size=256).astype(np.float32),
        "w_proj": (rng.standard_normal((256, 256)) * 0.05).astype(np.float32),
        "g_proj": rng.uniform(size=256).astype(np.float32),
        "b_proj": (rng.standard_normal(256) * 0.05).astype(np.float32),
        "m_proj": (rng.standard_normal(256) * 0.05).astype(np.float32),
        "var_proj": rng.uniform(size=256).astype(np.float32),
    }
    y = kernel(**ins)
    print("kernel ran, out shape", y.shape, "absmax", np.abs(y).max())

